# revision 1
# baseline (speedup 1.0000x reference)
"""Cumulative (causal) normalization kernel for TRN2, 8 NeuronCores.

x: [32, 512, 4000] f32.  out = (x - cum_mean) / sqrt(cum_var + eps), cumsum
along frames.  Data-parallel: rows = batch*bins flattened -> 16384 rows,
2048 rows per core.  Per 128-row x 2000-frame half-tile:

  xsq  = x^2                                  (ACT Square)
  s1   = cumsum(x)                            (DVE scan)
  s2e  = cumsum(xsq) + n*eps                  (DVE scan, data1=eps)
  t1   = x * n                                (Pool TT)
  num  = t1 - s1                              (DVE TT)
  t2   = s2e * n                              (DVE/Pool TT)
  t3   = s1^2                                 (ACT Square)
  W    = t2 - t3        (= n^2*(var+eps))     (DVE TT)
  r    = 1/sqrt(|W|)                          (ACT Abs_reciprocal_sqrt)
  out  = num * r                              (DVE TT)

The frame-chain is split across the two half-tiles by seeding the second
half's scans with the first half's final prefix values (scan initial=AP).
"""

import numpy as np

EPS = 1e-4
B, NBINS, F = 32, 512, 4000
P = 128
FD = 2000          # frames per half-tile
NCORES = 8
ROWS = B * NBINS               # 16384
ROWS_PER_CORE = ROWS // NCORES  # 2048
NT = ROWS_PER_CORE // P         # 16 row-tiles per core

_CACHE = {}


def _build():
    import concourse.bacc as bacc
    import concourse.mybir as mybir
    import concourse.tile as tile

    f32 = mybir.dt.float32
    nc = bacc.Bacc()

    x_d = nc.dram_tensor("x", [ROWS_PER_CORE, F], f32, kind="ExternalInput")
    n_d = nc.dram_tensor("nmul", [P, F], f32, kind="ExternalInput")
    i_d = nc.dram_tensor("idpm", [P, 2 * P], f32, kind="ExternalInput")
    o_d = nc.dram_tensor("out", [ROWS_PER_CORE, F], f32, kind="ExternalOutput")

    add = mybir.AluOpType.add
    byp = mybir.AluOpType.bypass
    SQ = mybir.ActivationFunctionType.Square
    ARS = mybir.ActivationFunctionType.Abs_reciprocal_sqrt

    with tile.TileContext(nc) as tc:
        with (
            tc.tile_pool(name="cst", bufs=1) as cst,
            tc.tile_pool(name="io", bufs=2) as io,
            tc.tile_pool(name="io2", bufs=2) as io2,
            tc.tile_pool(name="wk", bufs=2) as wk,
            tc.tile_pool(name="wx", bufs=1) as wx,
            tc.tile_pool(name="pp", bufs=2, space="PSUM") as pp,
        ):
            nmul = cst.tile([P, F], f32)
            nc.sync.dma_start(out=nmul, in_=n_d[:, :])
            idpm = cst.tile([P, 2 * P], f32)
            nc.sync.dma_start(out=idpm, in_=i_d[:, :])
            idt = idpm[:, 0:P]
            nid = idpm[:, P:2 * P]
            epst = cst.tile([P, FD], f32)
            nc.vector.memset(epst, EPS)

            for it in range(NT):
                r0 = it * P
                x_t = io.tile([P, F], f32, tag="x")
                nc.sync.dma_start(out=x_t, in_=x_d[r0:r0 + P, :])
                out_t = io2.tile([P, F], f32, tag="o")

                t1f = wk.tile([P, F], f32, tag="t1f")
                nc.vector.tensor_mul(t1f, x_t, nmul)

                prev_s1 = None
                prev_s2e = None
                for h in range(2):
                    lo = h * FD
                    hi = lo + FD
                    xs = x_t[:, lo:hi]
                    ns = nmul[:, lo:hi]

                    xsq = wx.tile([P, FD], f32, tag="xsq")
                    nc.scalar.activation(xsq, xs, SQ)

                    s1 = wk.tile([P, FD], f32, tag="s1")
                    nc.vector.tensor_tensor_scan(
                        out=s1, data0=xs, data1=xs,
                        initial=(0.0 if h == 0 else prev_s1[:, FD - 1:FD]),
                        op0=add, op1=byp)

                    s2e = wk.tile([P, FD], f32, tag="s2e")
                    nc.vector.tensor_tensor_scan(
                        out=s2e, data0=xsq, data1=epst,
                        initial=(0.0 if h == 0 else prev_s2e[:, FD - 1:FD]),
                        op0=add, op1=add)
                    prev_s1, prev_s2e = s1, s2e


                    t2 = wk.tile([P, FD], f32, tag="t2")
                    nc.vector.tensor_mul(t2, s2e, ns)

                    t3 = wx.tile([P, FD], f32, tag="t3")
                    nc.scalar.activation(t3, s1, SQ)

                    for q0 in range(0, FD, 1024):
                        qw = min(1024, FD - q0)
                        pnum = pp.tile([P, 1024], f32, tag="pnum")
                        pw = pp.tile([P, 1024], f32, tag="pw")
                        for c0 in range(q0, q0 + qw, 512):
                            c1 = min(c0 + 512, q0 + qw)
                            d0, d1 = c0 - q0, c1 - q0
                            nc.tensor.matmul(pnum[:, d0:d1], idt,
                                             t1f[:, lo + c0:lo + c1],
                                             start=True, stop=False)
                            nc.tensor.matmul(pw[:, d0:d1], idt,
                                             t2[:, c0:c1],
                                             start=True, stop=False)
                            nc.tensor.matmul(pnum[:, d0:d1], nid,
                                             s1[:, c0:c1],
                                             start=False, stop=True)
                            nc.tensor.matmul(pw[:, d0:d1], nid,
                                             t3[:, c0:c1],
                                             start=False, stop=True)

                        r = wx.tile([P, 1024], f32, tag="r")
                        nc.scalar.activation(r[:, 0:qw], pw[:, 0:qw], ARS)
                        nc.vector.tensor_mul(
                            out_t[:, lo + q0:lo + q0 + qw],
                            pnum[:, 0:qw], r[:, 0:qw])

                nc.sync.dma_start(out=o_d[r0:r0 + P, :], in_=out_t)

    nc.finalize()
    return nc


def kernel(x: np.ndarray) -> np.ndarray:
    from concourse import bass_utils

    assert x.shape == (B, NBINS, F) and x.dtype == np.float32
    if "nc" not in _CACHE:
        _CACHE["nc"] = _build()
    nc = _CACHE["nc"]

    nmul = np.broadcast_to(
        np.arange(1, F + 1, dtype=np.float32)[None, :], (P, F)
    ).copy()
    idpm = np.concatenate(
        [np.eye(P, dtype=np.float32), -np.eye(P, dtype=np.float32)], axis=1
    )

    xf = np.ascontiguousarray(x.reshape(ROWS, F))
    in_maps = [
        {"x": xf[c * ROWS_PER_CORE:(c + 1) * ROWS_PER_CORE], "nmul": nmul,
         "idpm": idpm}
        for c in range(NCORES)
    ]
    res = bass_utils.run_bass_kernel_spmd(nc, in_maps, core_ids=list(range(NCORES)))
    out = np.concatenate([r["out"] for r in res.results], axis=0)
    return out.reshape(B, NBINS, F)



# revision 2
# speedup vs baseline: 2.8625x; 2.8625x over previous
"""Cumulative (causal) normalization kernel for TRN2, 8 NeuronCores.

x: [32, 512, 4000] f32.  out = (x - cum_mean) / sqrt(cum_var + eps), cumsum
along frames.  Data parallel: 16384 rows over 8 cores, 16 row-tiles of
[128, 4000] per core.  Per tile (n = 1..4000 along frames):

  xsq  = x^2                       ACT Square
  S1   = cumsum(x)                 DVE scan (4000-wide, one instr)
  S2e  = cumsum(xsq) + n*eps       DVE scan (data1 = eps)
  t1fn = x * (-n)                  Pool TT
  t2n  = S2e * (-n)                Pool TT
  t3   = S1^2                      ACT Square
  pnum = I@t1fn + I@S1             PE psum accumulate  (= s1 - n*x = -num)
  pW   = I@t2n + I@t3              PE psum accumulate  (= s1^2 - n*s2e = -W)
  r    = ARS(pW)                   ACT (1/sqrt|W|)
  out  = (pnum * -1) * r           DVE scalar_tensor_tensor (psum read)

Single stationary (identity) for every matmul: signs are folded into the
operands, so the PE never reloads weights between matmuls.  The B-phase
(t1fn/t2n/t3/matmul/r/out) is chunked at 1000 cols and software-pipelined
one tile behind the A-phase (DMA/xsq/scans) to keep all engines busy.
"""

import numpy as np

EPS = 1e-4
B, NBINS, F = 32, 512, 4000
P = 128
NCORES = 8
ROWS = B * NBINS                  # 16384
ROWS_PER_CORE = ROWS // NCORES    # 2048
NT = ROWS_PER_CORE // P           # 16 row-tiles per core
CHUNK = 1000                      # B-phase chunk (2 PSUM banks per tile)
NCH = F // CHUNK

_CACHE = {}


def _build():
    import concourse.bacc as bacc
    import concourse.mybir as mybir
    import concourse.tile as tile

    f32 = mybir.dt.float32
    nc = bacc.Bacc()

    x_d = nc.dram_tensor("x", [ROWS_PER_CORE, F], f32, kind="ExternalInput")
    n_d = nc.dram_tensor("negn", [P, F], f32, kind="ExternalInput")
    i_d = nc.dram_tensor("ident", [P, P], f32, kind="ExternalInput")
    o_d = nc.dram_tensor("out", [ROWS_PER_CORE, F], f32, kind="ExternalOutput")

    add = mybir.AluOpType.add
    byp = mybir.AluOpType.bypass
    mult = mybir.AluOpType.mult
    SQ = mybir.ActivationFunctionType.Square
    ARS = mybir.ActivationFunctionType.Abs_reciprocal_sqrt

    with tile.TileContext(nc) as tc:
        with (
            tc.tile_pool(name="cst", bufs=1) as cst,
            tc.tile_pool(name="io", bufs=2) as io,
            tc.tile_pool(name="sc", bufs=2) as sc,
            tc.tile_pool(name="wa", bufs=1) as wa,
            tc.tile_pool(name="wb", bufs=2) as wb,
            tc.tile_pool(name="pp", bufs=2, space="PSUM") as pp,
        ):
            negn = cst.tile([P, F], f32)
            nc.sync.dma_start(out=negn, in_=n_d[:, :])
            idt = cst.tile([P, P], f32)
            nc.sync.dma_start(out=idt, in_=i_d[:, :])
            epst = cst.tile([P, F], f32)
            nc.vector.memset(epst, EPS)

            def phase_a(it):
                r0 = it * P
                x_t = io.tile([P, F], f32, tag="x")
                nc.sync.dma_start(out=x_t, in_=x_d[r0:r0 + P, :])
                xsq = wa.tile([P, F], f32, tag="xsq")
                nc.scalar.activation(xsq, x_t, SQ)
                s1 = sc.tile([P, F], f32, tag="s1")
                nc.vector.tensor_tensor_scan(
                    out=s1, data0=x_t, data1=x_t,
                    initial=0.0, op0=add, op1=byp)
                s2e = sc.tile([P, F], f32, tag="s2e")
                nc.vector.tensor_tensor_scan(
                    out=s2e, data0=xsq, data1=epst,
                    initial=0.0, op0=add, op1=add)
                return x_t, s1, s2e

            def phase_b(it, x_t, s1, s2e):
                r0 = it * P
                for c in range(NCH):
                    a = c * CHUNK
                    b = a + CHUNK
                    t1fn = wb.tile([P, CHUNK], f32, tag="t1fn")
                    nc.gpsimd.tensor_mul(t1fn, x_t[:, a:b], negn[:, a:b])
                    t2n = wb.tile([P, CHUNK], f32, tag="t2n")
                    nc.gpsimd.tensor_mul(t2n, s2e[:, a:b], negn[:, a:b])
                    t3 = wb.tile([P, CHUNK], f32, tag="t3")
                    nc.scalar.activation(t3, s1[:, a:b], SQ)

                    pnum = pp.tile([P, CHUNK], f32, tag="pnum")
                    pW = pp.tile([P, CHUNK], f32, tag="pW")
                    for q in (0, 512):
                        w = min(512, CHUNK - q)
                        nc.tensor.matmul(pnum[:, q:q + w], idt,
                                         t1fn[:, q:q + w],
                                         start=True, stop=False)
                        nc.tensor.matmul(pnum[:, q:q + w], idt,
                                         s1[:, a + q:a + q + w],
                                         start=False, stop=True)
                        nc.tensor.matmul(pW[:, q:q + w], idt,
                                         t2n[:, q:q + w],
                                         start=True, stop=False)
                        nc.tensor.matmul(pW[:, q:q + w], idt,
                                         t3[:, q:q + w],
                                         start=False, stop=True)

                    r_c = wb.tile([P, CHUNK], f32, tag="r")
                    nc.scalar.activation(r_c, pW, ARS)
                    out_c = wb.tile([P, CHUNK], f32, tag="o")
                    nc.vector.scalar_tensor_tensor(
                        out=out_c, in0=pnum, scalar=-1.0, in1=r_c,
                        op0=mult, op1=mult)
                    nc.sync.dma_start(out=o_d[r0:r0 + P, a:b], in_=out_c)

            prev = None
            for it in range(NT):
                cur = (it, *phase_a(it))
                if prev is not None:
                    phase_b(*prev)
                prev = cur
            phase_b(*prev)

    nc.finalize()
    return nc


def kernel(x: np.ndarray) -> np.ndarray:
    from concourse import bass_utils

    assert x.shape == (B, NBINS, F) and x.dtype == np.float32
    if "nc" not in _CACHE:
        _CACHE["nc"] = _build()
    nc = _CACHE["nc"]

    negn = np.broadcast_to(
        -np.arange(1, F + 1, dtype=np.float32)[None, :], (P, F)
    ).copy()
    ident = np.eye(P, dtype=np.float32)

    xf = np.ascontiguousarray(x.reshape(ROWS, F))
    in_maps = [
        {"x": xf[c * ROWS_PER_CORE:(c + 1) * ROWS_PER_CORE],
         "negn": negn, "ident": ident}
        for c in range(NCORES)
    ]
    res = bass_utils.run_bass_kernel_spmd(nc, in_maps, core_ids=list(range(NCORES)))
    out = np.concatenate([r["out"] for r in res.results], axis=0)
    return out.reshape(B, NBINS, F)
